# revision 1
# baseline (speedup 1.0000x reference)
"""Trainium2 Bass kernel for a GRU-like recurrent cell (4 unrolled timesteps)
with relu candidate and final output projection.

Math (per batch row, h0 = 0):
  for t in 0..3:
    r = sigmoid(x_t @ wr + h @ Ur + br)        # skipped at t=0 (r*h = 0)
    z = sigmoid(x_t @ wz + h @ Uz + bz)
    c = relu  (x_t @ wh + (r*h) @ Uh + bh)
    h = (1-z)*c + z*h
  y = relu(h @ w_out + b_out)

Distribution: data-parallel over batch across 8 cores (x/y sharded on dim 0,
weights replicated). Each core computes B_LOC=1024 rows.

Layout strategy (per core): all recurrent state is kept TRANSPOSED in SBUF as
[h_partition, batch_free] tiles, so the h @ U recurrence needs no transposes
(U tiles in natural layout are the stationary matmul operand, hT tiles are the
moving operand), gate biases become per-partition scalars for the ACT engine,
and the final projection uses hT tiles as the stationary operand producing the
output in natural [batch, unit] layout for direct DMA out.

x is the only tensor needing a transpose: it is cast fp32->bf16 into a DRAM
scratch ([T, B, D], SWDGE cast DMA), then loaded transposed via the hardware
xbar DMA transpose (2-byte dtype) as [d_partition, batch] tiles.

All matmul operands are bf16 (1 PE cycle/row vs 4 for fp32) with fp32 PSUM
accumulation. Weights are streamed from HBM each timestep (SWDGE cast
fp32->bf16 on load) to fit SBUF.
"""
import os
import numpy as np

B_FULL, T, D, H, U = 8192, 4, 2048, 1024, 2048
N_CORES = 8
B_LOC = B_FULL // N_CORES   # 1024
BC = 512                    # batch columns per moving-operand chunk
NBC = B_LOC // BC           # 2
KD = D // 128               # 16 contraction tiles for x @ W
KH = H // 128               # 8 contraction tiles for h @ U
NUC = U // BC               # 4 output column chunks
NBI = BC // 128             # 4 output row tiles per chunk

W_BUFS = 19
U_BUFS = 9
X_BUFS = 33
H_BUFS = 18
Z_BUFS = 16
RH_BUFS = 17


def _build():
    import concourse.mybir as mybir
    import concourse.tile as tile
    import concourse.bass as bass
    from concourse import bacc

    f32 = mybir.dt.float32
    bf16 = mybir.dt.bfloat16
    Act = mybir.ActivationFunctionType
    Alu = mybir.AluOpType

    def sl(i, step=128):
        return slice(i * step, (i + 1) * step)

    nc = bacc.Bacc("TRN2", target_bir_lowering=False, name="gru_cell")

    x_in = nc.dram_tensor("x", [B_LOC, T, D], f32, kind="ExternalInput")
    w_in = {
        "r": nc.dram_tensor("wr", [D, H], f32, kind="ExternalInput"),
        "z": nc.dram_tensor("wz", [D, H], f32, kind="ExternalInput"),
        "c": nc.dram_tensor("wh", [D, H], f32, kind="ExternalInput"),
    }
    u_in = {
        "r": nc.dram_tensor("Ur", [H, H], f32, kind="ExternalInput"),
        "z": nc.dram_tensor("Uz", [H, H], f32, kind="ExternalInput"),
        "c": nc.dram_tensor("Uh", [H, H], f32, kind="ExternalInput"),
    }
    b_in = {
        "r": nc.dram_tensor("br", [H], f32, kind="ExternalInput"),
        "z": nc.dram_tensor("bz", [H], f32, kind="ExternalInput"),
        "c": nc.dram_tensor("bh", [H], f32, kind="ExternalInput"),
    }
    wout_in = nc.dram_tensor("w_out", [H, U], f32, kind="ExternalInput")
    bout_in = nc.dram_tensor("b_out", [U], f32, kind="ExternalInput")
    y_out = nc.dram_tensor("y", [B_LOC, U], f32, kind="ExternalOutput")
    xbf = nc.dram_tensor("xbf", [T, B_LOC, D], bf16)
    # bf16 staging copies of the weights so steady-state streaming runs on
    # HWDGE (sync engine) instead of serializing on the gpsimd Q7 SWDGE path
    wbf = {g: nc.dram_tensor(f"wbf_{g}", [D, H], bf16) for g in ("r", "z", "c")}
    ubf = {g: nc.dram_tensor(f"ubf_{g}", [H, H], bf16) for g in ("r", "z", "c")}
    woutbf = nc.dram_tensor("woutbf", [H, U], bf16)

    with tile.TileContext(nc) as tc:
        with tc.tile_pool(name="sb", bufs=1) as sb, \
             tc.tile_pool(name="ps", bufs=6, space="PSUM") as ps:

            # per-partition gate biases: [128, KH], column j = bias[h_tile j]
            bias_sb = {}
            for g in ("r", "z", "c"):
                bt = sb.tile([128, KH], f32, name=f"bias_{g}", tag=f"bias_{g}")
                nc.sync.dma_start(bt, b_in[g].ap().rearrange("(kh p) -> p kh", p=128))
                bias_sb[g] = bt
            # output bias broadcast across partitions: [128, U]
            bout_ap = bout_in.ap()
            bout_bcast_src = bass.AP(
                tensor=bout_ap.tensor, offset=bout_ap.offset,
                ap=[[0, 128]] + list(bout_ap.ap))
            bout_sb = sb.tile([128, U], bf16, name="bout_sb", tag="bout_sb")
            nc.gpsimd.dma_start(bout_sb, bout_bcast_src)

            # x cast pipeline, off the Q7/SWDGE path: HWDGE load fp32
            # [128,1024] -> ACT cast bf16 -> HWDGE store to xbf, then xbar
            # transpose loads. Pipe and xbars are emitted separately so the
            # xbars (which wait on xt slot recycling) can be placed late in
            # the sync queue while the pipe runs early.
            xts_all = {}

            def emit_x_pipe_quad(t, bc, half):
                for blk in range(4):
                    b0 = bc * BC + blk * 128
                    xs32 = sb.tile([128, 1024], f32,
                                   name=f"xs32_t{t}b{bc}h{half}k{blk}",
                                   tag="xs32", bufs=2)
                    nc.sync.dma_start(
                        xs32, x_in[b0:b0 + 128, t, sl(half, 1024)])
                    xs16 = sb.tile([128, 1024], bf16,
                                   name=f"xs16_t{t}b{bc}h{half}k{blk}",
                                   tag="xs16", bufs=2)
                    nc.scalar.copy(xs16, xs32)
                    nc.sync.dma_start(
                        xbf[t, b0:b0 + 128, sl(half, 1024)], xs16)

            def emit_xbars_quad(t, bc, half):
                xts = xts_all.setdefault(t, {})
                for kd in range(half * 8, half * 8 + 8):
                    xt_t = sb.tile([128, BC], bf16,
                                   name=f"xt_t{t}b{bc}k{kd}", tag="xt",
                                   bufs=X_BUFS)
                    nc.sync.dma_start(
                        xt_t, xbf[t, sl(bc, BC), sl(kd)], transpose=True)
                    xts[(bc, kd)] = xt_t

            def emit_x_pipe(t):
                for bc in range(NBC):
                    for half in range(2):
                        emit_x_pipe_quad(t, bc, half)

            def emit_xbars(t):
                for bc in range(NBC):
                    for half in range(2):
                        emit_xbars_quad(t, bc, half)

            # t=0 prologue: Wz tiles direct (SWDGE cast fp32->bf16, Q7 is
            # otherwise idle) + the x(t=0) cast pipeline on HWDGE/ACT
            wtiles = {}
            for kd in range(KD):
                wt = sb.tile([128, H], bf16, name=f"w_z{kd}_t0",
                             tag="w", bufs=W_BUFS)
                nc.gpsimd.dma_start(wt, w_in["z"][sl(kd), :])
                wtiles[("z", kd)] = wt
            for bc in range(NBC):      # t0: interleave pipe + xbars tightly
                for half in range(2):
                    emit_x_pipe_quad(0, bc, half)
                    emit_xbars_quad(0, bc, half)

            def emit_weight_copies():
                # DRAM->DRAM fp32->bf16 casts, in first-needed order
                # (t1-r streams direct from fp32, so wr/Ur copies can go last)
                nc.gpsimd.dma_start(wbf["z"][:, :], w_in["z"][:, :])
                nc.gpsimd.dma_start(ubf["z"][:, :], u_in["z"][:, :])
                nc.gpsimd.dma_start(wbf["c"][:, :], w_in["c"][:, :])
                nc.gpsimd.dma_start(ubf["c"][:, :], u_in["c"][:, :])
                nc.gpsimd.dma_start(wbf["r"][:, :], w_in["r"][:, :])
                nc.gpsimd.dma_start(ubf["r"][:, :], u_in["r"][:, :])
                nc.gpsimd.dma_start(woutbf[:, :], wout_in[:, :])

            h = {}     # (kh, bc) -> bf16 [128, BC] hidden state, transposed
            utiles = {}

            for t in range(T):
                rh = {}
                z = {}
                xts = xts_all[t]
                stages = ("r", "z", "c") if t > 0 else ("z", "c")
                for g in stages:
                    # Q7 ordering: the weight staging copies go after t1-r's
                    # direct loads (t0 + t1-r stream straight from fp32 via
                    # SWDGE; later stages stream bf16 copies via HWDGE)
                    if t == 1 and g == "z":
                        emit_weight_copies()
                    if t < T - 1 and g == "c":
                        emit_x_pipe(t + 1)
                    direct = (t == 0) or (t == 1 and g == "r")
                    # stream this gate's weights (t=0 z came from prologue)
                    if t > 0 or g == "c":
                        for kd in range(KD):
                            wt = sb.tile([128, H], bf16,
                                         name=f"w_{g}{kd}_t{t}", tag="w",
                                         bufs=W_BUFS)
                            if direct:
                                nc.gpsimd.dma_start(wt, w_in[g][sl(kd), :])
                            else:
                                nc.sync.dma_start(wt, wbf[g][sl(kd), :])
                            wtiles[(g, kd)] = wt
                    if t > 0:
                        for kh in range(KH):
                            ut = sb.tile([128, H], bf16,
                                         name=f"u_{g}{kh}_t{t}", tag="u",
                                         bufs=U_BUFS)
                            if direct:
                                nc.gpsimd.dma_start(ut, u_in[g][sl(kh), :])
                            else:
                                nc.sync.dma_start(ut, ubf[g][sl(kh), :])
                            utiles[(g, kh)] = ut

                    for bc in range(NBC):
                        for ht in range(KH):
                            p = ps.tile([128, BC], f32,
                                        name=f"p_{g}_t{t}b{bc}h{ht}", tag="ps")
                            nmm = KD + (KH if t > 0 else 0)
                            i = 0
                            for kd in range(KD):
                                nc.tensor.matmul(
                                    p, wtiles[(g, kd)][:, sl(ht)],
                                    xts[(bc, kd)],
                                    start=(i == 0), stop=(i == nmm - 1))
                                i += 1
                            if t > 0:
                                rhs_map = rh if g == "c" else h
                                for kh in range(KH):
                                    nc.tensor.matmul(
                                        p, utiles[(g, kh)][:, sl(ht)],
                                        rhs_map[(kh, bc)],
                                        start=False, stop=(i == nmm - 1))
                                    i += 1

                            if g == "r":
                                # r kept fp32: bf16 resolution near 1.0 is
                                # 2^-8 which wrecks saturated gates
                                rt = sb.tile([128, BC], f32,
                                             name=f"r_t{t}b{bc}h{ht}",
                                             tag="r", bufs=4)
                                nc.scalar.activation(
                                    rt, p, Act.Sigmoid,
                                    bias=bias_sb["r"][:, ht:ht + 1])
                                rh_t = sb.tile([128, BC], bf16,
                                               name=f"rh_t{t}b{bc}h{ht}",
                                               tag="rh", bufs=RH_BUFS)
                                nc.vector.tensor_mul(rh_t, rt, h[(ht, bc)])
                                rh[(ht, bc)] = rh_t
                            elif g == "z":
                                zt = sb.tile([128, BC], f32,
                                             name=f"z_t{t}b{bc}h{ht}",
                                             tag="z", bufs=Z_BUFS)
                                nc.scalar.activation(
                                    zt, p, Act.Sigmoid,
                                    bias=bias_sb["z"][:, ht:ht + 1])
                                z[(ht, bc)] = zt
                            else:  # candidate + h update
                                hc = sb.tile([128, BC], bf16,
                                             name=f"hc_t{t}b{bc}h{ht}",
                                             tag="hc", bufs=4)
                                nc.scalar.activation(
                                    hc, p, Act.Relu,
                                    bias=bias_sb["c"][:, ht:ht + 1])
                                h_new = sb.tile([128, BC], bf16,
                                                name=f"h_t{t}b{bc}h{ht}",
                                                tag="h", bufs=H_BUFS)
                                if t == 0:
                                    # h1 = (1-z)*hc = hc - z*hc
                                    e = sb.tile([128, BC], f32,
                                                name=f"e_t{t}b{bc}h{ht}",
                                                tag="tmp1", bufs=3)
                                    nc.vector.tensor_mul(e, z[(ht, bc)], hc)
                                    nc.vector.tensor_sub(h_new, hc, e)
                                else:
                                    # h' = hc + z*(h - hc)
                                    d_ = sb.tile([128, BC], f32,
                                                 name=f"d_t{t}b{bc}h{ht}",
                                                 tag="tmp1", bufs=3)
                                    nc.vector.tensor_sub(d_, h[(ht, bc)], hc)
                                    e = sb.tile([128, BC], f32,
                                                name=f"e_t{t}b{bc}h{ht}",
                                                tag="tmp2", bufs=3)
                                    nc.vector.tensor_mul(e, z[(ht, bc)], d_)
                                    nc.vector.tensor_add(h_new, e, hc)
                                h[(ht, bc)] = h_new
                    # (end bc loop)
                # xbars for t+1 go at the end of t's sync-queue emissions so
                # their xt-slot waits can't block this step's weight streams
                if t < T - 1:
                    emit_xbars(t + 1)

            # final projection: y = relu(hT.T @ w_out + b_out)
            # w_out streamed per u-half as 8 tiles [128, 1024], "w" slots
            for half in range(2):
                wo = {}
                for kh in range(KH):
                    wt = sb.tile([128, H], bf16, name=f"wo_{kh}_{half}",
                                 tag="w", bufs=W_BUFS)
                    nc.sync.dma_start(wt, woutbf[sl(kh), sl(half, 1024)])
                    wo[kh] = wt
                for uc in (2 * half, 2 * half + 1):
                    for bc in range(NBC):
                        for bi in range(NBI):
                            p = ps.tile([128, BC], f32,
                                        name=f"po_b{bc}i{bi}u{uc}", tag="ps")
                            for kh in range(KH):
                                nc.tensor.matmul(
                                    p, h[(kh, bc)][:, sl(bi)],
                                    wo[kh][:, sl(uc % 2, 512)],
                                    start=(kh == 0), stop=(kh == KH - 1))
                            ot = sb.tile([128, BC], f32,
                                         name=f"ot_b{bc}i{bi}u{uc}",
                                         tag="otmp", bufs=2)
                            nc.vector.tensor_add(ot, p,
                                                 bout_sb[:, sl(uc, BC)])
                            oo = sb.tile([128, BC], f32,
                                         name=f"oo_b{bc}i{bi}u{uc}",
                                         tag="o", bufs=2)
                            nc.scalar.activation(oo, ot, Act.Relu)
                            nc.sync.dma_start(
                                y_out[bc * BC + bi * 128:
                                      bc * BC + (bi + 1) * 128,
                                      sl(uc, BC)], oo)

    nc.finalize()
    return nc


_nc_cache = None


def _get_nc():
    global _nc_cache
    if _nc_cache is None:
        _nc_cache = _build()
    return _nc_cache


def run(inputs, trace=False):
    """Run on 8 cores; returns (y_full, BassKernelResults)."""
    from concourse.bass_utils import run_bass_kernel_spmd

    nc = _get_nc()
    arrs = {k: np.ascontiguousarray(np.asarray(v, dtype=np.float32))
            for k, v in inputs.items()}
    in_maps = []
    for c in range(N_CORES):
        m = {k: v for k, v in arrs.items() if k != "x"}
        m["x"] = np.ascontiguousarray(arrs["x"][c * B_LOC:(c + 1) * B_LOC])
        in_maps.append(m)
    res = run_bass_kernel_spmd(nc, in_maps, core_ids=list(range(N_CORES)),
                               trace=trace)
    y = np.concatenate([res.results[c]["y"] for c in range(N_CORES)], axis=0)
    return y.astype(np.float32), res


def kernel(**inputs) -> np.ndarray:
    y, _ = run(inputs, trace=False)
    return y



# revision 4
# speedup vs baseline: 1.5048x; 1.5048x over previous
"""Trainium2 Bass kernel for a GRU-like recurrent cell (4 unrolled timesteps)
with relu candidate and final output projection.

Math (per batch row, h0 = 0):
  for t in 0..3:
    r = sigmoid(x_t @ wr + h @ Ur + br)        # skipped at t=0 (r*h = 0)
    z = sigmoid(x_t @ wz + h @ Uz + bz)
    c = relu  (x_t @ wh + (r*h) @ Uh + bh)
    h = (1-z)*c + z*h
  y = relu(h @ w_out + b_out)

Distribution: data-parallel over batch across 8 cores (x/y sharded on dim 0,
weights replicated). Each core computes B_LOC=1024 rows.

v2: fp8 (e4m3) DoubleRow matmuls for the error-tolerant sites (x@wr, x@wz,
h@Ur, h@Uz, (r*h)@Uh) at 2 contraction elements/partition/cycle; bf16 for the
error-critical sites (x@wh, h@w_out).  Scales: weights x256, U matrices x32,
h-state x8 -> every gate PSUM accumulates 256*(true preactivation), dequantized
for free by the ACT engine (out = func(psum*(1/256) + bias)).  wh is scaled
x256 in bf16 so the mixed bf16+fp8 accumulation shares one PSUM scale.

z is stored as wbar = 1-z = sigmoid(-pre) in bf16: saturated gates (z ~ 1,
driven by the positive-mean h @ Uz sum) need relative precision on 1-z, not z.
h update: h' = h - wbar*(h - hc); t=0: h1 = wbar*hc.

All recurrent state is kept TRANSPOSED in SBUF as [h_partition, batch_free]
tiles.  x is cast fp32->bf16 into a DRAM scratch then loaded transposed via
the 2-byte xbar DMA transpose; fp8 copies of the transposed x tiles are made
by ACT casts in SBUF (xbar cannot transpose 1-byte data).

Weights are loaded fp32 ONCE, cast+scaled on-chip (DVE), stored to DRAM
staging in the exact packed pair-tile layout ([128, 2, 1024] fp8: sub-tile i
holds contraction rows 128i..128i+127 of a 256-row pair block), and streamed
per gate-stage from there.  wh is packed per output-column-block (ht) so the
candidate stage only keeps 2 of 8 ht windows resident.
"""
import numpy as np

B_FULL, T, D, H, U = 8192, 4, 2048, 1024, 2048
N_CORES = 8
B_LOC = B_FULL // N_CORES   # 1024
BC = 512                    # batch columns per moving-operand chunk
NBC = B_LOC // BC           # 2
KD = D // 128               # 16 contraction tiles for x @ W
KDP = KD // 2               # 8 fp8 pair tiles
KH = H // 128               # 8 contraction tiles for h @ U
KHP = KH // 2               # 4 fp8 pair tiles
NUC = U // BC               # 4 output column chunks
NBI = BC // 128             # 4 output row tiles per chunk

SW = 256.0                  # weight scale (wr, wz, wh)
SU = 32.0                   # U matrix scale
SH = 8.0                    # h state scale  (SW = SU * SH)

# tag slot counts (tuned to fit 224KB/partition SBUF)
S32_BUFS = 3
XS16_BUFS = 2
XT16_BUFS = 33     # 32 hard-live in c stage (16 kd x 2 bc) + 1
XT8_BUFS = 16
W8_BUFS = 17
WHP_BUFS = 6
WH16N_BUFS = 3
H_BUFS = 18
H8_BUFS = 10
RH8_BUFS = 8       # all 8 (4 khp x 2 bc) live through c stage
WBAR_BUFS = 17     # all 16 (8 ht x 2 bc) live into c stage + 1
R_BUFS = 4
HC_BUFS = 3
DE_BUFS = 3


def _build():
    import concourse.mybir as mybir
    import concourse.tile as tile
    import concourse.bass as bass
    from concourse import bacc

    f32 = mybir.dt.float32
    bf16 = mybir.dt.bfloat16
    fp8 = mybir.dt.float8e4
    Act = mybir.ActivationFunctionType
    DR = mybir.MatmulPerfMode.DoubleRow

    def sl(i, step=128):
        return slice(i * step, (i + 1) * step)

    nc = bacc.Bacc("TRN2", target_bir_lowering=False, name="gru_fp8")

    x_in = nc.dram_tensor("x", [B_LOC, T, D], f32, kind="ExternalInput")
    w_in = {
        "r": nc.dram_tensor("wr", [D, H], f32, kind="ExternalInput"),
        "z": nc.dram_tensor("wz", [D, H], f32, kind="ExternalInput"),
        "c": nc.dram_tensor("wh", [D, H], f32, kind="ExternalInput"),
    }
    u_in = {
        "r": nc.dram_tensor("Ur", [H, H], f32, kind="ExternalInput"),
        "z": nc.dram_tensor("Uz", [H, H], f32, kind="ExternalInput"),
        "c": nc.dram_tensor("Uh", [H, H], f32, kind="ExternalInput"),
    }
    b_in = {
        "r": nc.dram_tensor("br", [H], f32, kind="ExternalInput"),
        "z": nc.dram_tensor("bz", [H], f32, kind="ExternalInput"),
        "c": nc.dram_tensor("bh", [H], f32, kind="ExternalInput"),
    }
    wout_in = nc.dram_tensor("w_out", [H, U], f32, kind="ExternalInput")
    bout_in = nc.dram_tensor("b_out", [U], f32, kind="ExternalInput")
    y_out = nc.dram_tensor("y", [B_LOC, U], f32, kind="ExternalOutput")
    xbf = nc.dram_tensor("xbf", [T, B_LOC, D], bf16)
    # packed fp8 pair-tile staging: [kdp][part 128][sub 2][col 1024]
    w8s = {g: nc.dram_tensor(f"w8s_{g}", [KDP, 128, 2, H], fp8)
           for g in ("r", "z")}
    u8s = {g: nc.dram_tensor(f"u8s_{g}", [KHP, 128, 2, H], fp8)
           for g in ("r", "z", "c")}
    # wh packed per ht: [ht 8][kd 16][part 128][col 128] bf16 (x256)
    whp_s = nc.dram_tensor("whp_s", [KH, KD, 128, 128], bf16)

    with tile.TileContext(nc) as tc:
        with tc.tile_pool(name="sb", bufs=1) as sb, \
             tc.tile_pool(name="ps", bufs=6, space="PSUM") as ps:

            # ---- biases: [128, KH] per-partition scalars per h-tile ----
            bias_sb = {}
            for g in ("r", "z", "c"):
                bt = sb.tile([128, KH], f32, name=f"bias_{g}", tag=f"bias_{g}")
                nc.sync.dma_start(bt, b_in[g].ap().rearrange("(kh p) -> p kh", p=128))
                bias_sb[g] = bt
            # negated bz for wbar = sigmoid(-pre - bz)
            bzn_sb = sb.tile([128, KH], f32, name="bzn", tag="bzn")
            nc.vector.tensor_scalar_mul(bzn_sb, bias_sb["z"], -1.0)
            # output bias broadcast across partitions: [128, U]
            bout_ap = bout_in.ap()
            bout_bcast_src = bass.AP(
                tensor=bout_ap.tensor, offset=bout_ap.offset,
                ap=[[0, 128]] + list(bout_ap.ap))
            bout_sb = sb.tile([128, U], bf16, name="bout_sb", tag="bout_sb")
            nc.gpsimd.dma_start(bout_sb, bout_bcast_src)

            # ---- x cast pipeline (fp32 -> bf16 DRAM scratch -> xbar) ----
            xts_all = {}   # (t) -> {(bc, kd): bf16 [128, 512] tile}
            xt8_all = {}   # (t) -> {(bc, kdp): fp8 [128, 2, 512] tile}

            def emit_x_pipe_quad(t, bc, half):
                for blk in range(4):
                    b0 = bc * BC + blk * 128
                    xs32 = sb.tile([128, 1024], f32,
                                   name=f"xs32_t{t}b{bc}h{half}k{blk}",
                                   tag="s32", bufs=S32_BUFS)
                    nc.sync.dma_start(
                        xs32, x_in[b0:b0 + 128, t, sl(half, 1024)])
                    xs16 = sb.tile([128, 1024], bf16,
                                   name=f"xs16_t{t}b{bc}h{half}k{blk}",
                                   tag="xs16", bufs=XS16_BUFS)
                    nc.scalar.copy(xs16, xs32)
                    nc.sync.dma_start(
                        xbf[t, b0:b0 + 128, sl(half, 1024)], xs16)

            def emit_xbars_quad(t, bc, half):
                xts = xts_all.setdefault(t, {})
                for kd in range(half * 8, half * 8 + 8):
                    xt_t = sb.tile([128, BC], bf16,
                                   name=f"xt_t{t}b{bc}k{kd}", tag="xt",
                                   bufs=XT16_BUFS)
                    nc.sync.dma_start(
                        xt_t, xbf[t, sl(bc, BC), sl(kd)], transpose=True)
                    xts[(bc, kd)] = xt_t

            def emit_x_pipe(t):
                for bc in range(NBC):
                    for half in range(2):
                        emit_x_pipe_quad(t, bc, half)

            def emit_xbars(t):
                for bc in range(NBC):
                    for half in range(2):
                        emit_xbars_quad(t, bc, half)

            def emit_xt8(t):
                # ACT casts bf16 -> fp8 pair tiles [128, 2, 512]
                xts = xts_all[t]
                x8 = xt8_all.setdefault(t, {})
                for bc in range(NBC):
                    for kdp in range(KDP):
                        t8 = sb.tile([128, 2, BC], fp8,
                                     name=f"xt8_t{t}b{bc}p{kdp}", tag="xt8",
                                     bufs=XT8_BUFS)
                        for i in range(2):
                            nc.scalar.copy(t8[:, i, :], xts[(bc, 2 * kdp + i)])
                        x8[(bc, kdp)] = t8

            # ---- one-time weight cast + pack-store pipelines ----
            def emit_w8_cast(g, retain=False):
                # w[g] fp32 [D, H] -> pair tiles [128, 2, H] fp8 x SW
                tiles = []
                for kdp in range(KDP):
                    w8t = sb.tile([128, 2, H], fp8, name=f"w8c_{g}{kdp}",
                                  tag="w8", bufs=W8_BUFS)
                    for i in range(2):
                        s32 = sb.tile([128, 1024], f32,
                                      name=f"wc32_{g}{kdp}i{i}",
                                      tag="s32", bufs=S32_BUFS)
                        nc.sync.dma_start(s32, w_in[g][sl(2 * kdp + i), :])
                        nc.vector.tensor_scalar_mul(w8t[:, i, :], s32, SW)
                    nc.sync.dma_start(w8s[g][kdp], w8t)
                    if retain:
                        tiles.append(w8t)
                return tiles

            def emit_u8_cast(g):
                for khp in range(KHP):
                    u8t = sb.tile([128, 2, H], fp8, name=f"u8c_{g}{khp}",
                                  tag="w8", bufs=W8_BUFS)
                    for i in range(2):
                        s32 = sb.tile([128, 1024], f32,
                                      name=f"uc32_{g}{khp}i{i}",
                                      tag="s32", bufs=S32_BUFS)
                        nc.sync.dma_start(s32, u_in[g][sl(2 * khp + i), :])
                        nc.vector.tensor_scalar_mul(u8t[:, i, :], s32, SU)
                    nc.sync.dma_start(u8s[g][khp], u8t)

            def emit_wh_cast():
                # wh fp32 [D, H] -> x256 bf16, packed [ht][kd][128][128]
                whp_ap = whp_s.ap()
                for kd in range(KD):
                    s32 = sb.tile([128, 1024], f32, name=f"whc32_{kd}",
                                  tag="s32", bufs=S32_BUFS)
                    nc.sync.dma_start(s32, w_in["c"][sl(kd), :])
                    w16 = sb.tile([128, KH, 128], bf16, name=f"wh16n_{kd}",
                                  tag="wh16n", bufs=WH16N_BUFS)
                    nc.vector.tensor_scalar_mul(w16, s32, SW)
                    # store (p, ht, m) -> whp_s[ht, kd, p, m]
                    dst = bass.AP(
                        tensor=whp_ap.tensor,
                        offset=whp_ap.offset + kd * 128 * 128,
                        ap=[[128, 128], [KD * 128 * 128, KH], [1, 128]])
                    nc.sync.dma_start(dst, w16)

            # ---- streamed loads ----
            def load_w8(g, t):
                tiles = []
                for kdp in range(KDP):
                    w8t = sb.tile([128, 2, H], fp8, name=f"w8_{g}{kdp}_t{t}",
                                  tag="w8", bufs=W8_BUFS)
                    nc.sync.dma_start(w8t, w8s[g][kdp])
                    tiles.append(w8t)
                return tiles

            def load_u8(g, t):
                tiles = []
                for khp in range(KHP):
                    u8t = sb.tile([128, 2, H], fp8, name=f"u8_{g}{khp}_t{t}",
                                  tag="w8", bufs=W8_BUFS)
                    nc.sync.dma_start(u8t, u8s[g][khp])
                    tiles.append(u8t)
                return tiles

            def load_whp(ht, t):
                # 2 tiles of [128, 8, 128] covering kd 0-7 / 8-15 for one ht
                whp_ap = whp_s.ap()
                tiles = []
                for j in range(2):
                    wt = sb.tile([128, 8, 128], bf16,
                                 name=f"whp_t{t}h{ht}j{j}", tag="whp",
                                 bufs=WHP_BUFS)
                    src = bass.AP(
                        tensor=whp_ap.tensor,
                        offset=whp_ap.offset
                        + ht * KD * 128 * 128 + j * 8 * 128 * 128,
                        ap=[[128, 128], [128 * 128, 8], [1, 128]])
                    nc.sync.dma_start(wt, src)
                    tiles.append(wt)
                return tiles

            # ---- recurrent state ----
            h = {}      # (kh, bc) -> bf16 [128, BC] hidden state (transposed)
            h8 = {}     # (khp, bc) -> fp8 [128, 2, BC], value = 8*h
            wz8_t0 = emit_w8_cast("z", retain=True)   # t0 z uses casts direct
            emit_wh_cast()

            # t0 x pipe + xbars, tightly interleaved, then fp8 casts
            for bc in range(NBC):
                for half in range(2):
                    emit_x_pipe_quad(0, bc, half)
                    emit_xbars_quad(0, bc, half)
            emit_xt8(0)

            # one-time casts for later steps (ordered by first need)
            emit_u8_cast("z")
            emit_u8_cast("c")
            emit_w8_cast("r")
            emit_u8_cast("r")

            for t in range(T):
                xts = xts_all[t]
                x8 = xt8_all[t]
                rh8 = {}
                wbar = {}

                # ---------- r stage (t >= 1) ----------
                if t > 0:
                    w8r = load_w8("r", t) if t > 1 else w8r_first
                    u8r = load_u8("r", t) if t > 1 else u8r_first
                    r_tiles = {}
                    for ht in range(KH):
                        for bc in range(NBC):
                            p = ps.tile([128, BC], f32,
                                        name=f"pr_t{t}b{bc}h{ht}", tag="ps")
                            n = KDP + KHP
                            i = 0
                            for kdp in range(KDP):
                                nc.tensor.matmul(
                                    p, w8r[kdp][:, :, sl(ht)], x8[(bc, kdp)],
                                    start=(i == 0), stop=(i == n - 1),
                                    perf_mode=DR)
                                i += 1
                            for khp in range(KHP):
                                nc.tensor.matmul(
                                    p, u8r[khp][:, :, sl(ht)], h8[(khp, bc)],
                                    start=False, stop=(i == n - 1),
                                    perf_mode=DR)
                                i += 1
                            rt = sb.tile([128, BC], f32,
                                         name=f"r_t{t}b{bc}h{ht}",
                                         tag="r", bufs=R_BUFS)
                            nc.scalar.activation(
                                rt, p, Act.Sigmoid,
                                bias=bias_sb["r"][:, ht:ht + 1],
                                scale=1.0 / SW)
                            r_tiles[(ht, bc)] = rt
                            # rh8 pair tile: alloc at even ht, fill halves
                            khp_i, i_h = ht // 2, ht % 2
                            if i_h == 0:
                                rh8[(khp_i, bc)] = sb.tile(
                                    [128, 2, BC], fp8,
                                    name=f"rh8_t{t}b{bc}p{khp_i}",
                                    tag="rh8", bufs=RH8_BUFS)
                            nc.vector.tensor_mul(
                                rh8[(khp_i, bc)][:, i_h, :], rt,
                                h8[(ht // 2, bc)][:, ht % 2, :])

                # prefetch z weights were loaded before r for t==1 (see below)
                # ---------- z stage ----------
                if t == 0:
                    w8z, u8z = wz8_t0, None
                else:
                    w8z = load_w8("z", t)
                    u8z = load_u8("z", t)
                for ht in range(KH):
                    for bc in range(NBC):
                        p = ps.tile([128, BC], f32,
                                    name=f"pz_t{t}b{bc}h{ht}", tag="ps")
                        n = KDP + (KHP if t > 0 else 0)
                        i = 0
                        for kdp in range(KDP):
                            nc.tensor.matmul(
                                p, w8z[kdp][:, :, sl(ht)], x8[(bc, kdp)],
                                start=(i == 0), stop=(i == n - 1),
                                perf_mode=DR)
                            i += 1
                        if t > 0:
                            for khp in range(KHP):
                                nc.tensor.matmul(
                                    p, u8z[khp][:, :, sl(ht)], h8[(khp, bc)],
                                    start=False, stop=(i == n - 1),
                                    perf_mode=DR)
                                i += 1
                        wb = sb.tile([128, BC], bf16,
                                     name=f"wbar_t{t}b{bc}h{ht}",
                                     tag="wbar", bufs=WBAR_BUFS)
                        nc.scalar.activation(
                            wb, p, Act.Sigmoid,
                            bias=bzn_sb[:, ht:ht + 1], scale=-1.0 / SW)
                        wbar[(ht, bc)] = wb

                # mid-step: next step's x pipe (DMA-heavy, overlaps c MMs)
                if t < T - 1:
                    emit_x_pipe(t + 1)

                # ---------- c stage + h update (ht-outer for whp windows) ---
                u8c = load_u8("c", t) if t > 0 else None
                h_new = {}
                h8_new = {}
                whp_tiles = {0: load_whp(0, t), 1: load_whp(1, t)}
                for ht in range(KH):
                    if ht + 2 < KH:
                        whp_tiles[ht + 2] = load_whp(ht + 2, t)
                    wja, wjb = whp_tiles[ht]
                    for bc in range(NBC):
                        p = ps.tile([128, BC], f32,
                                    name=f"pc_t{t}b{bc}h{ht}", tag="ps")
                        n = KD + (KHP if t > 0 else 0)
                        i = 0
                        for kd in range(KD):
                            wt = wja if kd < 8 else wjb
                            nc.tensor.matmul(
                                p, wt[:, kd % 8, :], xts[(bc, kd)],
                                start=(i == 0), stop=(i == n - 1))
                            i += 1
                        if t > 0:
                            for khp in range(KHP):
                                nc.tensor.matmul(
                                    p, u8c[khp][:, :, sl(ht)],
                                    rh8[(khp, bc)],
                                    start=False, stop=(i == n - 1),
                                    perf_mode=DR)
                                i += 1
                        hc = sb.tile([128, BC], bf16,
                                     name=f"hc_t{t}b{bc}h{ht}",
                                     tag="hc", bufs=HC_BUFS)
                        nc.scalar.activation(
                            hc, p, Act.Relu,
                            bias=bias_sb["c"][:, ht:ht + 1], scale=1.0 / SW)
                        hn = sb.tile([128, BC], bf16,
                                     name=f"h_t{t}b{bc}h{ht}",
                                     tag="h", bufs=H_BUFS)
                        if t == 0:
                            # h1 = (1-z)*hc = wbar*hc
                            nc.vector.tensor_mul(hn, wbar[(ht, bc)], hc)
                        else:
                            # h' = h - wbar*(h - hc)
                            d_ = sb.tile([128, BC], f32,
                                         name=f"d_t{t}b{bc}h{ht}",
                                         tag="tmp1", bufs=DE_BUFS)
                            nc.vector.tensor_sub(d_, h[(ht, bc)], hc)
                            e_ = sb.tile([128, BC], f32,
                                         name=f"e_t{t}b{bc}h{ht}",
                                         tag="tmp2", bufs=DE_BUFS)
                            nc.vector.tensor_mul(e_, wbar[(ht, bc)], d_)
                            nc.vector.tensor_sub(hn, h[(ht, bc)], e_)
                        h_new[(ht, bc)] = hn
                        if t < T - 1:
                            # h8' = cast(h' * 8) into pair slot
                            khp_i, i_h = ht // 2, ht % 2
                            if i_h == 0:
                                h8_new[(khp_i, bc)] = sb.tile(
                                    [128, 2, BC], fp8,
                                    name=f"h8_t{t}b{bc}p{khp_i}",
                                    tag="h8", bufs=H8_BUFS)
                            nc.scalar.activation(
                                h8_new[(khp_i, bc)][:, i_h, :], hn,
                                Act.Copy, scale=SH)
                h = h_new
                h8 = h8_new

                # tail: next step's transposes + fp8 casts + r weights
                if t < T - 1:
                    emit_xbars(t + 1)
                    emit_xt8(t + 1)
                    if t == 0:
                        w8r_first = load_w8("r", 1)
                        u8r_first = load_u8("r", 1)

            # ---- final projection: y = relu(hT.T @ w_out + b_out) ----
            for half in range(2):
                wo = {}
                for kh in range(KH):
                    s32 = sb.tile([128, 1024], f32, name=f"wo32_{kh}_{half}",
                                  tag="s32", bufs=S32_BUFS)
                    nc.sync.dma_start(s32, wout_in[sl(kh), sl(half, 1024)])
                    wt = sb.tile([128, 1024], bf16, name=f"wo_{kh}_{half}",
                                 tag="w8", bufs=W8_BUFS)
                    nc.vector.tensor_copy(wt, s32)
                    wo[kh] = wt
                for uc in (2 * half, 2 * half + 1):
                    for bc in range(NBC):
                        for bi in range(NBI):
                            p = ps.tile([128, BC], f32,
                                        name=f"po_b{bc}i{bi}u{uc}", tag="ps")
                            for kh in range(KH):
                                nc.tensor.matmul(
                                    p, h[(kh, bc)][:, sl(bi)],
                                    wo[kh][:, sl(uc % 2, 512)],
                                    start=(kh == 0), stop=(kh == KH - 1))
                            ot = sb.tile([128, BC], f32,
                                         name=f"ot_b{bc}i{bi}u{uc}",
                                         tag="otmp", bufs=2)
                            nc.vector.tensor_add(ot, p,
                                                 bout_sb[:, sl(uc, BC)])
                            oo = sb.tile([128, BC], f32,
                                         name=f"oo_b{bc}i{bi}u{uc}",
                                         tag="o", bufs=2)
                            nc.scalar.activation(oo, ot, Act.Relu)
                            nc.sync.dma_start(
                                y_out[bc * BC + bi * 128:
                                      bc * BC + (bi + 1) * 128,
                                      sl(uc, BC)], oo)

    nc.finalize()
    return nc


_nc_cache = None


def _get_nc():
    global _nc_cache
    if _nc_cache is None:
        _nc_cache = _build()
    return _nc_cache


def run(inputs, trace=False):
    """Run on 8 cores; returns (y_full, BassKernelResults)."""
    from concourse.bass_utils import run_bass_kernel_spmd

    nc = _get_nc()
    arrs = {k: np.ascontiguousarray(np.asarray(v, dtype=np.float32))
            for k, v in inputs.items()}
    in_maps = []
    for c in range(N_CORES):
        m = {k: v for k, v in arrs.items() if k != "x"}
        m["x"] = np.ascontiguousarray(arrs["x"][c * B_LOC:(c + 1) * B_LOC])
        in_maps.append(m)
    res = run_bass_kernel_spmd(nc, in_maps, core_ids=list(range(N_CORES)),
                               trace=trace)
    y = np.concatenate([res.results[c]["y"] for c in range(N_CORES)], axis=0)
    return y.astype(np.float32), res


def kernel(**inputs) -> np.ndarray:
    y, _ = run(inputs, trace=False)
    return y
